# revision 9
# baseline (speedup 1.0000x reference)
import os
import sys

import numpy as np

if "/opt/trn_rl_repo" not in sys.path:
    sys.path.insert(0, "/opt/trn_rl_repo")

import concourse.bacc as bacc
import concourse.mybir as mybir
import concourse.tile as tile
from concourse import bass_utils
from concourse.masks import make_identity

N, M, ENC, ATTN = 1024, 1024, 512, 256
NCORES = 8
NSH = N // NCORES

J = 10
L = 4.8
C_LIN = 0.2083311960444676
B = [
    0.5369101223589892,
    0.17027370650558718,
    0.06001996267067354,
    0.021444214619098373,
    0.007654910110471499,
    0.0027501850243399698,
    0.0009731318055065914,
    0.00035718536705591864,
    0.00011998891353047706,
    4.950963541821239e-05,
]
BIG = float(3 * 2**22)
TWO_PI = float(2.0 * np.pi)
PI = float(np.pi)

F32 = mybir.dt.float32
U8 = mybir.dt.uint8
AX = mybir.AxisListType.X
ALU = mybir.AluOpType
ACTF = mybir.ActivationFunctionType


def _emit(nc, tc, ctx):
    q_d = nc.dram_tensor("q", [NSH, ENC], F32, kind="ExternalInput")
    k_d = nc.dram_tensor("k", [M, ENC], F32, kind="ExternalInput")
    v_d = nc.dram_tensor("v", [M, ENC], F32, kind="ExternalInput")
    mask_d = nc.dram_tensor("mask", [NSH, M], U8, kind="ExternalInput")
    Qw_d = nc.dram_tensor("Qw", [ATTN, ENC], F32, kind="ExternalInput")
    Qb_d = nc.dram_tensor("Qb", [ATTN], F32, kind="ExternalInput")
    Kw_d = nc.dram_tensor("Kw", [ATTN, ENC], F32, kind="ExternalInput")
    Kb_d = nc.dram_tensor("Kb", [ATTN], F32, kind="ExternalInput")
    Vw_d = nc.dram_tensor("Vw", [ATTN, ENC], F32, kind="ExternalInput")
    Vb_d = nc.dram_tensor("Vb", [ATTN], F32, kind="ExternalInput")
    Ww_d = nc.dram_tensor("Ww", [1, ATTN], F32, kind="ExternalInput")
    Wb_d = nc.dram_tensor("Wb", [1], F32, kind="ExternalInput")
    out_d = nc.dram_tensor("context", [NSH, ATTN], F32, kind="ExternalOutput")

    constp = ctx.enter_context(tc.tile_pool(name="constp", bufs=1))
    workps = ctx.enter_context(tc.tile_pool(name="workps", bufs=3, space="PSUM"))
    scorep = ctx.enter_context(tc.tile_pool(name="scorep", bufs=1, space="PSUM"))

    ident = constp.tile([128, 128], F32)
    make_identity(nc, ident[:])
    ones = constp.tile([128, 512], F32)
    nc.gpsimd.memset(ones[:], 1.0)
    pi2 = constp.tile([128, 1], F32)
    nc.gpsimd.memset(pi2[:], PI / 2)

    ww_col = constp.tile([128, 2], F32)
    nc.sync.dma_start(out=ww_col[:], in_=Ww_d.ap().rearrange("o (c p) -> p (o c)", p=128, o=1))
    qb_row = constp.tile([1, ATTN], F32)
    nc.sync.dma_start(out=qb_row[:], in_=Qb_d.ap().rearrange("(o a) -> o a", o=1))
    kb_row = constp.tile([1, ATTN], F32)
    nc.sync.dma_start(out=kb_row[:], in_=Kb_d.ap().rearrange("(o a) -> o a", o=1))
    vb_row = constp.tile([1, ATTN], F32)
    nc.sync.dma_start(out=vb_row[:], in_=Vb_d.ap().rearrange("(o a) -> o a", o=1))
    wb_scrap = constp.tile([1, 1], F32)
    nc.sync.dma_start(out=wb_scrap[:], in_=Wb_d.ap().rearrange("(o a) -> o a", o=1))

    cww = constp.tile([128, 256], F32)
    for c in range(2):
        nc.vector.tensor_scalar(
            out=cww[:, c * 128 : (c + 1) * 128],
            in0=ones[:, 0:128],
            scalar1=ww_col[:, c : c + 1],
            scalar2=float(C_LIN),
            op0=ALU.mult,
            op1=ALU.mult,
        )

    projsb_cm = tc.tile_pool(name="projsb", bufs=1)
    projsb = projsb_cm.__enter__()
    k_nat = projsb.tile([128, 8 * ENC], F32)
    nc.sync.dma_start(out=k_nat[:].rearrange("p (t e) -> p t e", t=8), in_=k_d.ap().rearrange("(t p) e -> p t e", p=128))
    q_nat = projsb.tile([128, ENC], F32)
    nc.sync.dma_start(out=q_nat[:], in_=q_d.ap())
    qw_nat = projsb.tile([128, 2 * ENC], F32)
    nc.sync.dma_start(out=qw_nat[:].rearrange("p (t e) -> p t e", t=2), in_=Qw_d.ap().rearrange("(t p) e -> p t e", p=128))
    kw_nat = projsb.tile([128, 2 * ENC], F32)
    nc.sync.dma_start(out=kw_nat[:].rearrange("p (t e) -> p t e", t=2), in_=Kw_d.ap().rearrange("(t p) e -> p t e", p=128))
    v_nat = projsb.tile([128, 8 * ENC], F32)
    nc.sync.dma_start(out=v_nat[:].rearrange("p (t e) -> p t e", t=8), in_=v_d.ap().rearrange("(t p) e -> p t e", p=128))
    vw_nat = projsb.tile([128, 2 * ENC], F32)
    nc.sync.dma_start(out=vw_nat[:].rearrange("p (t e) -> p t e", t=2), in_=Vw_d.ap().rearrange("(t p) e -> p t e", p=128))

    kT = projsb.tile([128, 4 * M], F32)
    vT = projsb.tile([128, 4 * M], F32)
    for src, dst in ((k_nat, kT), (v_nat, vT)):
        for ec in range(4):
            for g in range(2):
                ps = workps.tile([128, 512], F32, tag="ps")
                for t in range(4):
                    mb = g * 4 + t
                    nc.tensor.transpose(
                        ps[:, t * 128 : (t + 1) * 128],
                        src[:, mb * ENC + ec * 128 : mb * ENC + (ec + 1) * 128],
                        ident[:],
                    )
                nc.any.tensor_copy(
                    dst[:, ec * M + g * 512 : ec * M + (g + 1) * 512], ps[:]
                )

    qT = projsb.tile([128, 512], F32)
    ps = workps.tile([128, 512], F32, tag="ps")
    for ec in range(4):
        nc.tensor.transpose(
            ps[:, ec * 128 : (ec + 1) * 128],
            q_nat[:, ec * 128 : (ec + 1) * 128],
            ident[:],
        )
    nc.any.tensor_copy(qT[:], ps[:])

    qwT = projsb.tile([128, 4 * ATTN], F32)
    kwT = projsb.tile([128, 4 * ATTN], F32)
    vwT = projsb.tile([128, 4 * ATTN], F32)
    for src, dst in ((qw_nat, qwT), (kw_nat, kwT), (vw_nat, vwT)):
        for ec in range(4):
            ps = workps.tile([128, 512], F32, tag="ps")
            for t in range(2):
                nc.tensor.transpose(
                    ps[:, t * 128 : (t + 1) * 128],
                    src[:, t * ENC + ec * 128 : t * ENC + (ec + 1) * 128],
                    ident[:],
                )
            nc.any.tensor_copy(dst[:, ec * ATTN : ec * ATTN + 256], ps[:, 0:256])

    kpT = [constp.tile([128, M], F32, tag=f"kpT{c}", name=f"kpT{c}") for c in range(2)]
    for c in range(2):
        for mh in range(2):
            ps = workps.tile([128, 512], F32, tag="ps")
            for ec in range(4):
                nc.tensor.matmul(
                    ps[:],
                    lhsT=kwT[:, ec * ATTN + c * 128 : ec * ATTN + (c + 1) * 128],
                    rhs=kT[:, ec * M + mh * 512 : ec * M + (mh + 1) * 512],
                    start=(ec == 0),
                    stop=False,
                )
            nc.tensor.matmul(
                ps[:],
                lhsT=kb_row[0:1, c * 128 : (c + 1) * 128],
                rhs=ones[0:1, 0:512],
                start=False,
                stop=True,
            )
            nc.any.tensor_copy(kpT[c][:, mh * 512 : (mh + 1) * 512], ps[:])

    qpT = constp.tile([128, 256], F32)
    for c in range(2):
        ps = workps.tile([128, 512], F32, tag="ps")
        for ec in range(4):
            nc.tensor.matmul(
                ps[:, 0:128],
                lhsT=qwT[:, ec * ATTN + c * 128 : ec * ATTN + (c + 1) * 128],
                rhs=qT[:, ec * 128 : (ec + 1) * 128],
                start=(ec == 0),
                stop=False,
            )
        nc.tensor.matmul(
            ps[:, 0:128],
            lhsT=qb_row[0:1, c * 128 : (c + 1) * 128],
            rhs=ones[0:1, 0:128],
            start=False,
            stop=True,
        )
        nc.any.tensor_copy(qpT[:, c * 128 : (c + 1) * 128], ps[:, 0:128])

    vp = constp.tile([128, 8 * ATTN], F32)
    for b in range(8):
        ps = workps.tile([128, 512], F32, tag="ps")
        for ec in range(4):
            nc.tensor.matmul(
                ps[:, 0:256],
                lhsT=vT[:, ec * M + b * 128 : ec * M + (b + 1) * 128],
                rhs=vwT[:, ec * ATTN : (ec + 1) * ATTN],
                start=(ec == 0),
                stop=False,
            )
        nc.tensor.matmul(
            ps[:, 0:256],
            lhsT=ones[0:1, 0:128],
            rhs=vb_row[0:1, :],
            start=False,
            stop=True,
        )
        nc.any.tensor_copy(vp[:, b * ATTN : (b + 1) * ATTN], ps[:, 0:256])

    projsb_cm.__exit__(None, None, None)
    trigk = ctx.enter_context(tc.tile_pool(name="trigk", bufs=2))
    trigq = ctx.enter_context(tc.tile_pool(name="trigq", bufs=2))
    softp = ctx.enter_context(tc.tile_pool(name="softp", bufs=1))

    mask_u8 = softp.tile([128, M], U8)
    nc.sync.dma_start(out=mask_u8[:], in_=mask_d.ap())
    maskf = softp.tile([128, M], F32)
    nc.vector.tensor_copy(maskf[:], mask_u8[:])
    negm = softp.tile([128, M], F32)
    nc.vector.tensor_scalar(
        out=negm[:], in0=maskf[:], scalar1=1e6, scalar2=-1e6,
        op0=ALU.mult, op1=ALU.add,
    )

    scores = scorep.tile([128, M], F32)
    for j in range(1, J + 1):
        om_hat = float(j / (2.0 * L))
        bj = float(B[j - 1])

        Bq = trigq.tile([128, 256], F32, tag="Bq")
        nc.gpsimd.tensor_scalar(
            out=Bq[:], in0=qpT[:], scalar1=om_hat, scalar2=BIG,
            op0=ALU.mult, op1=ALU.add,
        )
        kq = trigq.tile([128, 256], F32, tag="kq")
        nc.gpsimd.tensor_scalar(
            out=kq[:], in0=Bq[:], scalar1=BIG, scalar2=None, op0=ALU.subtract,
        )
        usq = trigq.tile([128, 256], F32, tag="usq")
        nc.vector.scalar_tensor_tensor(
            out=usq[:], in0=qpT[:], scalar=om_hat, in1=kq[:],
            op0=ALU.mult, op1=ALU.subtract,
        )
        vcq = trigq.tile([128, 256], F32, tag="vcq")
        nc.vector.scalar_tensor_tensor(
            out=vcq[:], in0=usq[:], scalar=0.25, in1=usq[:],
            op0=ALU.is_ge, op1=ALU.subtract,
        )
        sq = trigq.tile([128, 256], F32, tag="sq")
        nc.scalar.activation(sq[:], usq[:], ACTF.Sin, bias=0.0, scale=TWO_PI)
        cq = trigq.tile([128, 256], F32, tag="cq")
        nc.scalar.activation(cq[:], vcq[:], ACTF.Sin, bias=pi2[:, 0:1], scale=-TWO_PI)
        Sq = trigq.tile([128, 256], F32, tag="Sq")
        Cq = trigq.tile([128, 256], F32, tag="Cq")
        for c in range(2):
            sl = slice(c * 128, (c + 1) * 128)
            nc.vector.tensor_scalar(
                out=Sq[:, sl], in0=sq[:, sl], scalar1=ww_col[:, c : c + 1],
                scalar2=bj, op0=ALU.mult, op1=ALU.mult,
            )
            nc.vector.tensor_scalar(
                out=Cq[:, sl], in0=cq[:, sl], scalar1=ww_col[:, c : c + 1],
                scalar2=bj, op0=ALU.mult, op1=ALU.mult,
            )

        for c in range(2):
            Bk = trigk.tile([128, M], F32, tag="Bk")
            nc.gpsimd.tensor_scalar(
                out=Bk[:], in0=kpT[c][:], scalar1=om_hat, scalar2=BIG,
                op0=ALU.mult, op1=ALU.add,
            )
            kk = trigk.tile([128, M], F32, tag="kk")
            nc.gpsimd.tensor_scalar(
                out=kk[:], in0=Bk[:], scalar1=BIG, scalar2=None, op0=ALU.subtract,
            )
            usk = trigk.tile([128, M], F32, tag="usk")
            nc.vector.scalar_tensor_tensor(
                out=usk[:], in0=kpT[c][:], scalar=om_hat, in1=kk[:],
                op0=ALU.mult, op1=ALU.subtract,
            )
            vck = trigk.tile([128, M], F32, tag="vck")
            nc.vector.scalar_tensor_tensor(
                out=vck[:], in0=usk[:], scalar=0.25, in1=usk[:],
                op0=ALU.is_ge, op1=ALU.subtract,
            )
            sk = trigk.tile([128, M], F32, tag="sk")
            nc.scalar.activation(sk[:], usk[:], ACTF.Sin, bias=0.0, scale=TWO_PI)
            ck = trigk.tile([128, M], F32, tag="ck")
            nc.scalar.activation(ck[:], vck[:], ACTF.Sin, bias=pi2[:, 0:1], scale=-TWO_PI)

            sl = slice(c * 128, (c + 1) * 128)
            for mh in range(2):
                msl = slice(mh * 512, (mh + 1) * 512)
                nc.tensor.matmul(
                    scores[:, msl], lhsT=Sq[:, sl], rhs=ck[:, msl],
                    start=(j == 1 and c == 0), stop=False,
                )
                nc.tensor.matmul(
                    scores[:, msl], lhsT=Cq[:, sl], rhs=sk[:, msl],
                    start=False, stop=False,
                )

    for c in range(2):
        for mh in range(2):
            msl = slice(mh * 512, (mh + 1) * 512)
            nc.tensor.matmul(
                scores[:, msl],
                lhsT=cww[:, c * 128 : (c + 1) * 128],
                rhs=kpT[c][:, msl],
                start=False,
                stop=(c == 1),
            )

    sm = softp.tile([128, M], F32)
    nc.vector.tensor_tensor(out=sm[:], in0=scores[:], in1=maskf[:], op=ALU.mult)
    sm2 = softp.tile([128, M], F32)
    nc.vector.tensor_tensor(out=sm2[:], in0=sm[:], in1=negm[:], op=ALU.add)

    negmx = softp.tile([128, 1], F32)
    nc.vector.tensor_reduce(
        out=negmx[:], in_=sm2[:], axis=AX, op=ALU.max, negate=True
    )
    ew = softp.tile([128, M], F32)
    nc.scalar.activation(ew[:], sm2[:], ACTF.Exp, bias=negmx[:, 0:1], scale=1.0)
    dsum = softp.tile([128, 1], F32)
    nc.vector.tensor_reduce(out=dsum[:], in_=ew[:], axis=AX, op=ALU.add)
    rinv = softp.tile([128, 1], F32)
    nc.vector.reciprocal(rinv[:], dsum[:])

    ewT = softp.tile([128, M], F32)
    for g in range(2):
        ps = workps.tile([128, 512], F32, tag="ps")
        for t in range(4):
            b = g * 4 + t
            nc.tensor.transpose(
                ps[:, t * 128 : (t + 1) * 128],
                ew[:, b * 128 : (b + 1) * 128],
                ident[:],
            )
        nc.any.tensor_copy(ewT[:, g * 512 : (g + 1) * 512], ps[:])

    ctx_ps = workps.tile([128, 256], F32, tag="ps")
    for b in range(8):
        nc.tensor.matmul(
            ctx_ps[:],
            lhsT=ewT[:, b * 128 : (b + 1) * 128],
            rhs=vp[:, b * ATTN : (b + 1) * ATTN],
            start=(b == 0),
            stop=(b == 7),
        )
    ctx_sb = softp.tile([128, ATTN], F32)
    nc.vector.tensor_scalar(
        out=ctx_sb[:], in0=ctx_ps[:], scalar1=rinv[:, 0:1], scalar2=None,
        op0=ALU.mult,
    )
    nc.sync.dma_start(out=out_d.ap(), in_=ctx_sb[:])


_CACHED = None


def build_nc():
    global _CACHED
    if _CACHED is not None:
        return _CACHED
    from contextlib import ExitStack

    nc = bacc.Bacc(
        "TRN2",
        debug=False,
        enable_asserts=False,
        target_bir_lowering=False,
        num_devices=NCORES,
    )
    with tile.TileContext(nc) as tc:
        with ExitStack() as ctx:
            _emit(nc, tc, ctx)
    nc.compile()
    _CACHED = nc
    return nc


def make_in_maps(q, k, v, mask, Qw, Qb, Kw, Kb, Vw, Vb, Ww, Wb):
    mask_u8 = np.ascontiguousarray(mask).view(np.uint8)
    shared = {
        "k": np.ascontiguousarray(k, np.float32),
        "v": np.ascontiguousarray(v, np.float32),
        "Qw": np.ascontiguousarray(Qw, np.float32),
        "Qb": np.ascontiguousarray(Qb, np.float32),
        "Kw": np.ascontiguousarray(Kw, np.float32),
        "Kb": np.ascontiguousarray(Kb, np.float32),
        "Vw": np.ascontiguousarray(Vw, np.float32),
        "Vb": np.ascontiguousarray(Vb, np.float32),
        "Ww": np.ascontiguousarray(Ww, np.float32),
        "Wb": np.ascontiguousarray(Wb, np.float32),
    }
    in_maps = []
    for c in range(NCORES):
        rows = slice(c * NSH, (c + 1) * NSH)
        in_maps.append(
            {
                "q": np.ascontiguousarray(q[rows], np.float32),
                "mask": np.ascontiguousarray(mask_u8[rows]),
                **shared,
            }
        )
    return in_maps


def kernel(**inputs) -> np.ndarray:
    nc = build_nc()
    in_maps = make_in_maps(**{k: np.asarray(v) for k, v in inputs.items()})
    res = bass_utils.run_bass_kernel_spmd(nc, in_maps, list(range(NCORES)))
    return np.concatenate([res.results[c]["context"] for c in range(NCORES)], axis=0)


if __name__ == "__main__":
    d = np.load("/tmp/inputs.npz")
    out = kernel(**{k: d[k] for k in d.files})
    print("kernel output", out.shape, out.dtype, float(np.abs(out).max()))
